# revision 34
# baseline (speedup 1.0000x reference)
"""Trainium2 Bass kernel for nn_MultiHeadSliddingWindowAttention.

The reference scatters the 3 sliding-window scores into COLUMNS 0..2 of the
[B,H,N,N] score tensor (faithful-to-source), then softmaxes over all N
columns.  Algebraically the whole attention collapses to, per (b, h, row i):

    out_i = (e0_i*V0 + e1_i*V1 + e2_i*V2 + C) / Z_i
    e_d   = exp(s_d),  s_0 = Q_i.K_{i-1}, s_1 = Q_i.K_i, s_2 = Q_i.K_{i+1}
            (s_d = 0 when the neighbour row does not exist)
    Z_i   = e0 + e1 + e2 + (N-3)
    V0..2 = first three rows of V;  C = sum_{j>=3} V_j

so the [N,N] score tensor never needs to be materialized.  Sharding: 8 cores
= 2 batches x 4 sequence chunks of 512 rows; each core computes Q/K for its
chunk (+1-row halo) and the full output projection for its rows.

v2: all matmuls bf16; output projection refactored via G = L @ Wo.T.
v3: inputs host-packed into a few [128, big] tensors (4KB+ descriptor lines).
v4: DMA issue spread over SP/GpSimd/Act rings; score matmuls interleave into
    the K loop; softmax tail split into column halves (separate PSUM banks).
v6: K bias folds into the K psum->SBUF cast (tensor_scalar_add) so the Q*K
    products are plain tensor_mul (HW runs TT at 2x, STT at 1x); one chunk's
    products go to GpSimd to unload DVE; reciprocal uses the fast custom-DVE
    approximation (~5x); fp8 was tried and REVERTED (real score range +-9
    makes exp amplify fp8 noise past the 2e-2 gate).
"""

import os
import numpy as np
import ml_dtypes

B, N, E = 2, 2048, 512
H, DQ = 8, 64
NCHUNK = 4           # sequence chunks per batch
CH = N // NCHUNK     # 512 rows per core
NCORES = 8
NM3 = float(N - 3)   # 2045

last_exec_time_ns = None
_prog = None


def _build_program():
    import concourse.bacc as bacc
    import concourse.mybir as mybir
    import concourse.tile as tile

    bf = mybir.dt.bfloat16
    f32 = mybir.dt.float32
    nc = bacc.Bacc(
        "TRN2",
        target_bir_lowering=False,
        debug=False,
        enable_asserts=False,
        num_devices=NCORES,
    )

    def din(name, shape, dt=bf):
        return nc.dram_tensor(name, shape, dt, kind="ExternalInput").ap()

    xt4 = din("xt4", [128, 4, 514])       # x.T halo chunk, k-subtiles
    wq4 = din("wq4", [128, 4, 512])       # Wq.T k-subtiles
    wk4 = din("wk4", [128, 4, 512])
    wv4 = din("wv4", [128, 4 * 512])
    wo4 = din("wo4", [128, 4 * 512])
    # packA: hsel [0:384] | xc4 k-blocks [384:512] | hmt m-blocks [512:640]
    packa = din("packa", [128, 640])
    # rowpack: bv [0:512] | bmul [512:544]
    rowp = din("rowp", [1, 544])
    # f32b: bqc m-blocks [0:4] | boc m-blocks [4:8] | bkc m-blocks [8:12]
    f32b = din("f32b", [128, 12], f32)
    blk = din("blk", [32, 32])            # block-diag Z reduction
    yt = nc.dram_tensor("yt", [512, 512], bf, kind="ExternalOutput").ap()

    with tile.TileContext(nc) as tc:
        _device_body(tc, mybir, bf, f32, xt4, wq4, wk4, wv4, wo4,
                     packa, rowp, f32b, blk, yt)
    nc.compile()
    return nc


def _device_body(tc, mybir, bf, f32, xt4, wq4, wk4, wv4, wo4,
                 packa, rowp, f32b, blk, yt):
    from contextlib import ExitStack

    nc = tc.nc
    with ExitStack() as ctx:
        const = ctx.enter_context(tc.tile_pool(name="const", bufs=1))
        work = ctx.enter_context(tc.tile_pool(name="work", bufs=6))
        psum = ctx.enter_context(tc.tile_pool(name="psum", bufs=3, space="PSUM"))
        psum2 = ctx.enter_context(tc.tile_pool(name="psum2", bufs=1, space="PSUM"))
        psum_s = ctx.enter_context(tc.tile_pool(name="psums", bufs=1, space="PSUM"))

        def load(eng, tag, src, shape, dt=bf):
            t = const.tile(shape, dt, tag=tag)
            eng.dma_start(out=t[...], in_=src)
            return t

        # Three DMA rings, each in its consumers' order.  wq4/xt4 lead their
        # rings (first matmul gates on them); f32b's 128 tiny descriptors go
        # on the Act ring so they don't stall wq4's generation.
        wqs = load(nc.sync, "wq4", wq4[:, :, :], [128, 4, 512])
        xts = load(nc.gpsimd, "xt4", xt4[:, :, :], [128, 4, 514])
        wks = load(nc.gpsimd, "wk4", wk4[:, :, :], [128, 4, 512])
        pas = load(nc.gpsimd, "packa", packa[:, :], [128, 640])
        rps = load(nc.gpsimd, "rowp", rowp[:, :], [1, 544])
        f32s = load(nc.scalar, "f32b", f32b[:, :], [128, 12], f32)
        # Gate the Act ring's bulk loads behind f32b so the first-needed
        # tensors get the DMA engines to themselves.
        gate = const.tile([1, 1], f32, tag="gate")
        nc.scalar.activation(gate[:, :], f32s[0:1, 0:1],
                             mybir.ActivationFunctionType.Identity)
        wvs = load(nc.scalar, "wv4", wv4[:, :], [128, 2048])
        wos = load(nc.scalar, "wo4", wo4[:, :], [128, 2048])
        blks = load(nc.scalar, "blk", blk[:, :], [32, 32])

        # PE warm-up: the tensor engine needs ~3us of continuous work before
        # it clocks up; run throwaway matmuls on a memset tile while the
        # input DMAs stream so the real Q matmuls start at full speed.
        warm = const.tile([32, 512], bf, tag="warm")
        nc.vector.memset(warm[:, :], 0.0)
        psw = psum_s.tile([32, 512], f32, tag="g", name="psw")
        for w in range(8):
            nc.tensor.matmul(psw[:, :], warm[:, 0:32], warm[:, :],
                             start=(w == 0), stop=False)
        # fine-grained tail keeps the PE continuously busy right up to the
        # moment xt4/wq4 land, so the p-state ramp carries into the real work
        for w in range(10):
            nc.tensor.matmul(psw[:, 0:128], warm[:, 0:32], warm[:, 0:128],
                             start=False, stop=(w == 9))

        wv = lambda k, m: wvs[:, 512 * k + 128 * m:512 * k + 128 * (m + 1)]
        wo = lambda m: wos[:, 512 * m:512 * (m + 1)]
        hsel = lambda idx: pas[:, 32 * idx:32 * (idx + 1)]
        xc = lambda k: pas[:, 384 + 32 * k:384 + 32 * (k + 1)]
        hmt = lambda m: pas[:, 512 + 32 * m:512 + 32 * (m + 1)]
        bv = lambda m: rps[0:1, 128 * m:128 * (m + 1)]
        bmul = rps[0:1, 512:544]
        bqc = lambda m: f32s[:, m:m + 1]
        boc = lambda m: f32s[:, 4 + m:4 + m + 1]
        bkc = lambda m: f32s[:, 8 + m:8 + m + 1]

        # ---- Q projection: Qt[m] = [128 ch_out, 512 rows] (bf16) ----
        qt_sb = []
        for m in range(4):
            ps = psum.tile([128, 512], f32, tag="mm")
            for k in range(4):
                nc.tensor.matmul(ps[:, :],
                                 wqs[:, k, 128 * m:128 * (m + 1)],
                                 xts[:, k, 1:513],
                                 start=(k == 0), stop=(k == 3))
            q = const.tile([128, 512], bf, tag=f"qt{m}")
            nc.scalar.activation(q[:, :], ps[:, :],
                                 mybir.ActivationFunctionType.Identity,
                                 bias=bqc(m))
            qt_sb.append(q)

        # ---- K projection; bias folds into the psum->SBUF cast ----
        # kt = K' + bk via tensor_scalar_add (pads get +bk instead of 0: the
        # resulting error is O(1e-5) on 2 of 4096 rows - negligible).
        # qk(m,d) = kt[d:d+512] * qt is a plain tensor_mul (2x on DVE); chunk
        # t=2's products run on GpSimd to unload DVE.
        pss_h = [psum_s.tile([32, 256], f32, tag="s0", name="pss0"),
                 psum_s.tile([32, 256], f32, tag="s1", name="pss1")]
        qk_sb = []

        def emit_scores(t):
            for h in range(2):
                c0, c1 = 256 * h, 256 * (h + 1)
                for d in range(3):
                    idx = 3 * t + d
                    nc.tensor.matmul(pss_h[h][:, :], hsel(idx),
                                     qk_sb[idx][:, c0:c1],
                                     start=(t == 0 and d == 0),
                                     stop=(t == 3 and d == 2))

        for m in range(4):
            kt = const.tile([128, 514], bf, tag=f"kt{m}")
            ps = psum.tile([128, 512], f32, tag="mm")
            for k in range(4):
                nc.tensor.matmul(ps[:, :],
                                 wks[:, k, 128 * m:128 * (m + 1)],
                                 xts[:, k, 0:512],
                                 start=(k == 0), stop=(k == 3))
            nc.vector.tensor_scalar_add(kt[:, 0:512], ps[:, :], bkc(m))
            ps2 = psum2.tile([128, 2], f32, tag="mm2")
            for k in range(4):
                nc.tensor.matmul(ps2[:, :],
                                 wks[:, k, 128 * m:128 * (m + 1)],
                                 xts[:, k, 512:514],
                                 start=(k == 0), stop=(k == 3))
            nc.vector.tensor_scalar_add(kt[:, 512:514], ps2[:, :], bkc(m))
            for d in range(3):
                qk = work.tile([128, 512], bf, tag="qk")
                nc.vector.tensor_mul(qk[:, :], kt[:, d:d + 512], qt_sb[m][:, :])
                qk_sb.append(qk)
            if m >= 1:
                emit_scores(m - 1)

        # last chunk's score matmuls (h0 first so exp h0 starts early)
        emit_scores(3)

        # ---- L.T via psvT: [128 ch, 32 (4h+d)]; then G = L @ Wo.T ----
        # Runs in the PE bubble while ACT computes exp of the scores.
        lt_sb = []
        for m in range(4):
            psv = psum2.tile([128, 32], f32, tag="mm2")
            for k in range(4):
                nc.tensor.matmul(psv[:, :], wv(k, m), xc(k),
                                 start=(k == 0), stop=False)
            nc.tensor.matmul(psv[:, :], bv(m), bmul, start=False, stop=True)
            lt = const.tile([128, 32], bf, tag=f"lt{m}")
            nc.vector.tensor_mul(lt[:, :], psv[:, :], hmt(m))
            lt_sb.append(lt)

        psg = psum_s.tile([32, 512], f32, tag="g")
        for m in range(4):
            nc.tensor.matmul(psg[:, :], lt_sb[m][:, :], wo(m),
                             start=(m == 0), stop=(m == 3))
        g_sb = const.tile([32, 512], bf, tag="g")
        nc.vector.tensor_copy(g_sb[:, :], psg[:, :])

        # ---- per column half: E = exp(S); Z = blk.T @ E; r = 1/Z; Eh = E*r --
        eh_half = []
        for h in range(2):
            e_sb = const.tile([32, 256], bf, tag=f"e{h}")
            nc.scalar.activation(e_sb[:, :], pss_h[h][:, :],
                                 mybir.ActivationFunctionType.Exp)
            psz = psum_s.tile([32, 256], f32, tag="z")
            nc.tensor.matmul(psz[:, :], blks[:, :], e_sb[:, :],
                             start=True, stop=True)
            r_sb = const.tile([32, 256], f32, tag=f"r{h}")
            nc.vector.reciprocal_approx_fast(r_sb[:, :], psz[:, :])
            eh = const.tile([32, 256], bf, tag=f"eh{h}")
            nc.vector.tensor_mul(eh[:, :], e_sb[:, :], r_sb[:, :])
            eh_half.append(eh)

        # ---- y[m] = G[:, m].T @ Ehat + bo ----
        # Bias alternates ACT/DVE so the four psum drains run on two engines.
        for m in range(4):
            psy = psum.tile([128, 512], f32, tag="mm")
            for h in range(2):
                c0, c1 = 256 * h, 256 * (h + 1)
                nc.tensor.matmul(psy[:, c0:c1], g_sb[:, 128 * m:128 * (m + 1)],
                                 eh_half[h][:, :], start=True, stop=True)
            y = work.tile([128, 512], bf, tag="y")
            if m % 2 == 0:
                nc.scalar.activation(y[:, :], psy[:, :],
                                     mybir.ActivationFunctionType.Identity,
                                     bias=boc(m))
            else:
                nc.vector.tensor_scalar_add(y[:, :], psy[:, :], boc(m))
            (nc.sync if m % 2 == 0 else nc.gpsimd).dma_start(
                out=yt[128 * m:128 * (m + 1), :], in_=y[:, :])


def _host_constants():
    # hsel consumed t-major: idx = 3t + d
    hsel = np.zeros((128, 384), np.float32)
    for t in range(4):
        for d in range(3):
            for p in range(128):
                m = 4 * (2 * t + p // 64) + d
                hsel[p, 32 * (3 * t + d) + m] = 1.0
    # hmt[ch, 4h+d] = 1 iff channel ch belongs to head h
    hmt = np.zeros((512, 32), np.float32)
    for ch in range(512):
        h = ch // 64
        hmt[ch, 4 * h:4 * h + 4] = 1.0
    blk = np.zeros((32, 32), np.float32)
    for k in range(32):
        for m in range(32):
            if k // 4 == m // 4:
                blk[k, m] = NM3 if k % 4 == 3 else 1.0
    return hsel, hmt, blk


def _bf(a):
    return np.ascontiguousarray(np.asarray(a).astype(ml_dtypes.bfloat16))


def _kblocks(a):
    """[512, F] -> [128, 4, F]: k-blocks of 128 rows stacked on dim 1."""
    F = a.shape[1]
    out = np.empty((128, 4, F), a.dtype)
    for k in range(4):
        out[:, k, :] = a[128 * k:128 * (k + 1), :]
    return out


def kernel(**inputs):
    global _prog, last_exec_time_ns
    from concourse.bass_utils import run_bass_kernel_spmd

    x = np.ascontiguousarray(np.asarray(inputs["x"], dtype=np.float32))
    wqt = np.asarray(inputs["Wq"], np.float32).T
    wkt = np.asarray(inputs["Wk"], np.float32).T
    wvt = np.asarray(inputs["Wv"], np.float32).T
    wot = np.asarray(inputs["Wo"], np.float32).T
    bq = np.asarray(inputs["bq"], np.float32)
    bo = np.asarray(inputs["bo"], np.float32)
    bkv = np.asarray(inputs["bk"], np.float32)
    bvv = np.asarray(inputs["bv"], np.float32)
    hsel, hmt, blk = _host_constants()

    wq4 = _bf(_kblocks(wqt))
    wk4 = _bf(_kblocks(wkt))
    wv4 = _bf(_kblocks(wvt).reshape(128, 2048))
    wo4 = _bf(_kblocks(wot).reshape(128, 2048))
    f32b = np.stack([bq[0:128], bq[128:256], bq[256:384], bq[384:512],
                     bo[0:128], bo[128:256], bo[256:384], bo[384:512],
                     bkv[0:128], bkv[128:256], bkv[256:384], bkv[384:512]],
                    axis=1).astype(np.float32)
    bmul = np.tile(np.array([1.0, 1.0, 1.0, NM3], np.float32), 8)
    rowp = _bf(np.concatenate([bvv, bmul]).reshape(1, 544))

    packa_b = []
    for b in range(B):
        cols = np.stack([x[b, 0], x[b, 1], x[b, 2], x[b, 3:].sum(0)], axis=1)
        xc4 = cols[:, np.tile(np.arange(4), 8)]             # [512, 32]
        pa = np.concatenate([hsel,
                             _kblocks(xc4.astype(np.float32)).reshape(128, 128),
                             _kblocks(hmt).reshape(128, 128)], axis=1)
        packa_b.append(_bf(pa))

    shared = {"wq4": wq4, "wk4": wk4, "wv4": wv4, "wo4": wo4,
              "f32b": f32b, "blk": _bf(blk), "rowp": rowp}
    in_maps = []
    for c in range(NCORES):
        b, j = divmod(c, NCHUNK)
        s = j * CH
        g0 = s - 1
        lo, hi = max(0, g0), min(N, s + CH + 1)
        xw = np.zeros((514, 512), np.float32)
        xw[lo - g0:hi - g0, :] = x[b, lo:hi, :]
        xt4 = _kblocks(np.ascontiguousarray(xw.T))          # [128, 4, 514]
        in_maps.append({"xt4": _bf(xt4), "packa": packa_b[b], **shared})

    if _prog is None:
        _prog = _build_program()

    trace = os.environ.get("KERNEL_TRACE", "0") == "1"
    try:
        res = run_bass_kernel_spmd(_prog, in_maps, list(range(NCORES)), trace=trace)
    except ModuleNotFoundError:
        # NTFF profiling hook unavailable in this axon client; run untraced.
        res = run_bass_kernel_spmd(_prog, in_maps, list(range(NCORES)), trace=False)
    last_exec_time_ns = res.exec_time_ns

    y = np.empty((B, N, E), np.float32)
    for c in range(NCORES):
        b, j = divmod(c, NCHUNK)
        y[b, j * CH:(j + 1) * CH, :] = res.results[c]["yt"].T.astype(np.float32)
    return y
